# revision 12
# baseline (speedup 1.0000x reference)
"""Trainium2 Bass kernel for the capsule-routing layer (nn_Caps_Layer).

Full inputs: x [32, 512, 768] f32, W [1, 768, 512] f32.
Output: [32, 16, 32] f32.

Strategy: data-parallel over batch across 8 NeuronCores (4 batches/core).
Host-side prep (free wrt device time): x is pre-transposed to d-major
[768, 512] per batch and cast to bf16; W cast to bf16. This removes all
on-device x transposes and halves HBM traffic.

Per core:
  - u[s, (n c)] tiles via PE bf16 matmuls from xT tiles (no transposes)
  - uT[(n c), s] tiles via PE transposes of u (bf16, 1 cyc/row)
  - 3 routing iterations fully on-chip with narrow (16-wide) matmuls that
    land results directly in the layout the next step needs:
      outputsT tile OFT[nc, n]  = sum_sc u_chunk^T @ cwT       (PE)
      bT[s, n]                  = sum_kc uT_chunk^T @ mblk     (PE)
    softmax over n on the free axis; squash's 1/sqrt via exp(-0.5*ln(x))
    so every ACT func lives in one activation table (exp/ln/copy/square)
    -> a single table load for the whole kernel.
  - final gather via a tiled-identity matrix; DMA out per batch.
"""
import numpy as np
import concourse.bass as bass
import concourse.mybir as mybir
import concourse.tile as tile
from concourse import bacc
from concourse.bass import ts, ds
from concourse.bass_utils import run_bass_kernel_spmd

F32 = mybir.dt.float32
BF16 = mybir.dt.bfloat16
AF = mybir.ActivationFunctionType
AX = mybir.AxisListType
OP = mybir.AluOpType

NCORES = 8
B, S, D = 32, 512, 768
N, C = 16, 32
NC = N * C            # 512
BL = B // NCORES      # 4 batches per core
EPS = 1e-7
SCN = S // 128        # 4 s-chunks
DCN = D // 128        # 6 d-chunks
KCN = NC // 128       # 4 nc-chunks
ROUTINGS = 3
NWARM = 28            # PE warm-up transposes (cover p-state ramp + DMA lead-in)


def _build_module():
    nc = bacc.Bacc("TRN2", target_bir_lowering=False, num_devices=NCORES)
    XT = nc.dram_tensor("xt", [BL, D, S], BF16, kind="ExternalInput")
    W = nc.dram_tensor("w", [D, NC], BF16, kind="ExternalInput")
    CB = nc.dram_tensor("cb", [128, 160], BF16, kind="ExternalInput")
    MK = nc.dram_tensor("mk", [128, 64], F32, kind="ExternalInput")
    OUT = nc.dram_tensor("out", [BL, N, C], F32, kind="ExternalOutput")

    ev_flip = [0]

    with tile.TileContext(nc) as tc:
        with (
            tc.tile_pool(name="const", bufs=1) as pc,
            tc.tile_pool(name="xp", bufs=3) as px_pool,
            tc.tile_pool(name="up", bufs=16) as pu_pool,
            tc.tile_pool(name="utp", bufs=16) as put_pool,
            tc.tile_pool(name="rt", bufs=6) as prt,
            tc.tile_pool(name="mmp", bufs=3, space="PSUM") as pmm,
            tc.tile_pool(name="trp", bufs=2, space="PSUM") as ptr,
            tc.tile_pool(name="rmp", bufs=3, space="PSUM") as prm,
        ):
            def evac(dst, src):
                # PSUM->SBUF evacuations alternate DVE/ACT (GPSIMD can't
                # read PSUM)
                if ev_flip[0] % 2 == 0:
                    nc.vector.tensor_copy(dst, src)
                else:
                    nc.scalar.copy(dst, src)
                ev_flip[0] += 1

            # ---- on-chip constants (no DMA) ----
            warm = pc.tile([128, 128], BF16, tag="warm")
            ones16b = pc.tile([128, 16], BF16, tag="ones16b")
            ones128b = pc.tile([128, 1], BF16, tag="ones128b")
            ones1xb = pc.tile([1, 128], BF16, tag="ones1xb")
            epst = pc.tile([1, 1], F32, tag="eps")
            nc.gpsimd.memset(warm[:], 0.0)
            nc.gpsimd.memset(ones16b[:], 1.0)
            nc.gpsimd.memset(ones128b[:], 1.0)
            nc.gpsimd.memset(ones1xb[:], 1.0)
            nc.gpsimd.memset(epst[:], EPS)

            # dummy Ln+Exp so the act-table pass settles on the
            # natural_log_exp table before any real activation
            dumt = prt.tile([1, 1], F32, tag="dum")
            nc.scalar.activation(dumt[:], epst[:], AF.Ln)
            nc.scalar.activation(dumt[:], dumt[:], AF.Exp)

            # PE warm-up: back-to-back dummy transposes keep the tensor
            # engine continuously busy through its p-state ramp while the
            # first x/W chunks stream in.
            wpsum = ptr.tile([128, S], BF16, tag="tr", name="wpsum")
            for _ in range(NWARM):
                nc.tensor.transpose(wpsum[:, 0:128], warm[:], warm[:])

            # ---- DMA'd constants ----
            identb = pc.tile([128, 128], BF16, tag="identb")
            gmatb = pc.tile([128, 32], BF16, tag="gmatb")
            masks = pc.tile([128, 64], F32, tag="masks")
            wsb = pc.tile([128, DCN, NC], BF16, tag="w")

            # ---- stage A: u and uT per batch ----
            # b0's x and W stream in 2-dc chunks, interleaved on one queue so
            # the first matmuls can start after ~2 chunks land.
            us = [[None] * SCN for _ in range(BL)]
            uts = [[None] * KCN for _ in range(BL)]
            xts = []
            for b in range(BL):
                xts.append(px_pool.tile([128, DCN, S], BF16, tag="x",
                                        name=f"xt{b}"))
            for h in range(3):
                nc.sync.dma_start(
                    wsb[:, 2 * h:2 * h + 2, :],
                    W[ds(256 * h, 256), :].rearrange("(dc p) n -> p dc n", p=128),
                )
                nc.sync.dma_start(
                    xts[0][:, 2 * h:2 * h + 2, :],
                    XT[0, ds(256 * h, 256), :].rearrange("(dc p) s -> p dc s", p=128),
                )
            nc.sync.dma_start(identb[:], CB[:, 0:128])
            nc.sync.dma_start(gmatb[:], CB[:, 128:160])
            nc.sync.dma_start(masks[:], MK[:, :])
            for b in range(1, BL):
                nc.sync.dma_start(
                    xts[b][:], XT[b, :, :].rearrange("(dc p) s -> p dc s", p=128)
                )

            for b in range(BL):
                # u tiles [128(s), 512(nc)]: accumulate over dc in dc-major
                # order so b0 consumes x/W chunks as they arrive
                for h in range(2):
                    scs = (2 * h, 2 * h + 1)
                    pus = {sc: pmm.tile([128, NC], F32, tag="mm",
                                        name=f"pu{b}_{sc}")
                           for sc in scs}
                    for dc in range(DCN):
                        for sc in scs:
                            nc.tensor.matmul(
                                pus[sc][:],
                                xts[b][:, dc, ts(sc, 128)],
                                wsb[:, dc, :],
                                start=(dc == 0),
                                stop=(dc == DCN - 1),
                            )
                    for sc in scs:
                        u = pu_pool.tile([128, NC], BF16, tag="u")
                        evac(u[:], pus[sc][:])
                        us[b][sc] = u
                # uT tiles [128(nc), 512(s)] via PE transposes (bf16)
                for kc in range(KCN):
                    put = ptr.tile([128, S], BF16, tag="tr")
                    for sc in range(SCN):
                        nc.tensor.transpose(
                            put[:, ts(sc, 128)],
                            us[b][sc][:, ts(kc, 128)],
                            identb[:],
                        )
                    ut = put_pool.tile([128, S], BF16, tag="ut")
                    evac(ut[:], put[:])
                    uts[b][kc] = ut

            # ---- routing ----
            # cwT[b]: [128(s), (sc n)=64] bf16 softmax weights (it>0)
            cwTs = [None] * BL
            for it in range(ROUTINGS):
                last = it == ROUTINGS - 1
                for b in range(BL):
                    # outputsT: OFT[nc, n] per kc chunk -> pot [128, (kc n)]
                    pot = prm.tile([128, 64], F32, tag="r", name=f"pot{it}_{b}")
                    rhs_full = ones16b if it == 0 else cwTs[b]
                    for kc in range(KCN):
                        for sc in range(SCN):
                            rhs = (rhs_full[:, 0:16] if it == 0
                                   else rhs_full[:, ts(sc, 16)])
                            nc.tensor.matmul(
                                pot[:, ts(kc, 16)],
                                us[b][sc][:, ts(kc, 128)],
                                rhs,
                                start=(sc == 0),
                                stop=(sc == SCN - 1),
                            )
                    # block-diag mask -> mraw; squared norms per capsule
                    mraw = prt.tile([128, 64], F32, tag="mraw")
                    nc.vector.tensor_mul(mraw[:], pot[:], masks[:])
                    sq = prt.tile([128, 64], BF16, tag="sq")
                    nc.scalar.activation(sq[:], mraw[:], AF.Square)
                    pnsq = prm.tile([1, 64], F32, tag="r", name="pnsq")
                    nc.tensor.matmul(pnsq[:], ones128b[:], sq[:],
                                     start=True, stop=True)
                    nred = prt.tile([1, 16], F32, tag="nred")
                    nc.vector.tensor_reduce(
                        nred[:],
                        pnsq[:].rearrange("o (k n) -> o n k", k=4),
                        axis=AX.X,
                        op=OP.add,
                    )
                    # 1/sqrt(nsq+eps) = exp(-0.5*ln(nsq+eps)); both funcs live
                    # in the same act table as softmax's exp
                    lnt = prt.tile([1, 16], F32, tag="lnt")
                    nc.scalar.activation(lnt[:], nred[:], AF.Ln, bias=epst[:])
                    invn = prt.tile([1, 16], BF16, tag="invn")
                    nc.scalar.activation(invn[:], lnt[:], AF.Exp, scale=-0.5)
                    pinv = prm.tile([128, 16], F32, tag="r", name="pinv")
                    nc.tensor.matmul(pinv[:], ones1xb[:], invn[:],
                                     start=True, stop=True)
                    # Mblk = mraw * invnorm (broadcast over the 4 kc groups)
                    pv = pinv[:, :]
                    inv_b = bass.AP(
                        tensor=pv.tensor,
                        offset=pv.offset,
                        ap=[pv.ap[0], [0, 4], [1, 16]],
                    )
                    mblk = prt.tile([128, 64], BF16, tag="mblk")
                    nc.vector.tensor_mul(
                        mblk[:].rearrange("p (k n) -> p k n", k=4),
                        mraw[:].rearrange("p (k n) -> p k n", k=4),
                        inv_b,
                    )
                    if not last:
                        # bT[s, n] per sc -> pbt [128, (sc n)]; softmax over n
                        pbt = prm.tile([128, 64], F32, tag="r",
                                       name=f"pbt{it}_{b}")
                        for sc in range(SCN):
                            for kc in range(KCN):
                                nc.tensor.matmul(
                                    pbt[:, ts(sc, 16)],
                                    uts[b][kc][:, ts(sc, 128)],
                                    mblk[:, ts(kc, 16)],
                                    start=(kc == 0),
                                    stop=(kc == KCN - 1),
                                )
                        expb = prt.tile([128, 64], F32, tag="expb")
                        nc.scalar.activation(expb[:], pbt[:], AF.Exp)
                        zsum = prt.tile([128, 4], F32, tag="zsum")
                        nc.vector.tensor_reduce(
                            zsum[:],
                            expb[:].rearrange("p (g n) -> p g n", g=4),
                            axis=AX.X,
                            op=OP.add,
                        )
                        zrec = prt.tile([128, 4], F32, tag="zrec")
                        nc.vector.reciprocal(zrec[:], zsum[:])
                        zr_ap = zrec[:, :]
                        zr_b = bass.AP(
                            tensor=zr_ap.tensor,
                            offset=zr_ap.offset,
                            ap=[zr_ap.ap[0], [1, 4], [0, 16]],
                        )
                        cwT = prt.tile([128, 64], BF16, tag="cw")
                        nc.vector.tensor_mul(
                            cwT[:].rearrange("p (g n) -> p g n", g=4),
                            expb[:].rearrange("p (g n) -> p g n", g=4),
                            zr_b,
                        )
                        cwTs[b] = cwT
                    else:
                        # final gather: pf[n, c] = sum_kc mblk_kc^T @ gmat
                        pf = prm.tile([16, 32], F32, tag="r", name="pf")
                        for kc in range(KCN):
                            nc.tensor.matmul(
                                pf[:],
                                mblk[:, ts(kc, 16)],
                                gmatb[:],
                                start=(kc == 0),
                                stop=(kc == KCN - 1),
                            )
                        fsb = prt.tile([16, 32], F32, tag="fin")
                        nc.vector.tensor_copy(fsb[:], pf[:])
                        out_eng = nc.sync if b % 2 == 0 else nc.scalar
                        out_eng.dma_start(OUT[b, :, :], fsb[:])

    nc.compile()
    return nc


def _make_consts():
    import ml_dtypes
    bf = ml_dtypes.bfloat16
    cb = np.zeros((128, 160), dtype=bf)
    cb[:, 0:128] = np.eye(128, dtype=np.float32).astype(bf)
    cb[:, 128:160] = np.tile(np.eye(32, dtype=np.float32), (4, 1)).astype(bf)
    masks = np.zeros((128, 64), dtype=np.float32)
    for k in range(4):
        for g in range(4):
            n = 4 * k + g
            masks[32 * g:32 * (g + 1), 16 * k + n] = 1.0
    return {"cb": cb, "mk": masks}


_NC_CACHE = []


def kernel(x: np.ndarray, W: np.ndarray) -> np.ndarray:
    import ml_dtypes
    bf = ml_dtypes.bfloat16
    assert x.shape == (B, S, D) and W.shape == (1, D, NC)
    if not _NC_CACHE:
        _NC_CACHE.append(_build_module())
    nc = _NC_CACHE[0]
    consts = _make_consts()
    w2 = np.ascontiguousarray(W[0]).astype(bf)
    in_maps = []
    for i in range(NCORES):
        m = dict(consts)
        xs = x[i * BL:(i + 1) * BL]
        m["xt"] = np.ascontiguousarray(xs.transpose(0, 2, 1)).astype(bf)
        m["w"] = w2
        in_maps.append(m)
    res = run_bass_kernel_spmd(nc, in_maps, list(range(NCORES)))
    out = np.concatenate([res.results[i]["out"] for i in range(NCORES)], axis=0)
    return out.astype(np.float32)


# revision 13
# speedup vs baseline: 1.2674x; 1.2674x over previous
"""Trainium2 Bass kernel for the capsule-routing layer (nn_Caps_Layer).

Full inputs: x [32, 512, 768] f32, W [1, 768, 512] f32.
Output: [32, 16, 32] f32.

Strategy: data-parallel over batch across 8 NeuronCores (4 batches/core).
Host-side prep (free wrt device time): x is pre-transposed to d-major
[768, 512] per batch and cast to bf16; W cast to bf16. This removes all
on-device x transposes and halves HBM traffic.

Per core:
  - u[s, (n c)] tiles via PE bf16 matmuls from xT tiles (no transposes)
  - uT[(n c), s] tiles via PE transposes of u (bf16, 1 cyc/row)
  - 3 routing iterations fully on-chip with narrow (16-wide) matmuls that
    land results directly in the layout the next step needs:
      outputsT tile OFT[nc, n]  = sum_sc u_chunk^T @ cwT       (PE)
      bT[s, n]                  = sum_kc uT_chunk^T @ mblk     (PE)
    softmax over n on the free axis; squash's 1/sqrt via exp(-0.5*ln(x))
    so every ACT func lives in one activation table (exp/ln/copy/square)
    -> a single table load for the whole kernel.
  - final gather via a tiled-identity matrix; DMA out per batch.
"""
import numpy as np
import concourse.bass as bass
import concourse.mybir as mybir
import concourse.tile as tile
from concourse import bacc
from concourse.bass import ts, ds
from concourse.bass_utils import run_bass_kernel_spmd

F32 = mybir.dt.float32
BF16 = mybir.dt.bfloat16
AF = mybir.ActivationFunctionType
AX = mybir.AxisListType
OP = mybir.AluOpType

NCORES = 8
B, S, D = 32, 512, 768
N, C = 16, 32
NC = N * C            # 512
BL = B // NCORES      # 4 batches per core
EPS = 1e-7
SCN = S // 128        # 4 s-chunks
DCN = D // 128        # 6 d-chunks
KCN = NC // 128       # 4 nc-chunks
ROUTINGS = 3
NWARM = 28            # PE warm-up transposes (cover p-state ramp + DMA lead-in)


def _build_module():
    nc = bacc.Bacc("TRN2", target_bir_lowering=False, num_devices=NCORES)
    XT = nc.dram_tensor("xt", [BL, D, S], BF16, kind="ExternalInput")
    W = nc.dram_tensor("w", [D, NC], BF16, kind="ExternalInput")
    CB = nc.dram_tensor("cb", [128, 160], BF16, kind="ExternalInput")
    MK = nc.dram_tensor("mk", [128, 64], F32, kind="ExternalInput")
    OUT = nc.dram_tensor("out", [BL, N, C], F32, kind="ExternalOutput")

    ev_flip = [0]

    with tile.TileContext(nc) as tc:
        with (
            tc.tile_pool(name="const", bufs=1) as pc,
            tc.tile_pool(name="xp", bufs=3) as px_pool,
            tc.tile_pool(name="up", bufs=16) as pu_pool,
            tc.tile_pool(name="utp", bufs=16) as put_pool,
            tc.tile_pool(name="rt", bufs=6) as prt,
            tc.tile_pool(name="mmp", bufs=3, space="PSUM") as pmm,
            tc.tile_pool(name="trp", bufs=2, space="PSUM") as ptr,
            tc.tile_pool(name="rmp", bufs=3, space="PSUM") as prm,
        ):
            def evac(dst, src):
                # PSUM->SBUF evacuations alternate DVE/ACT (GPSIMD can't
                # read PSUM)
                if ev_flip[0] % 2 == 0:
                    nc.vector.tensor_copy(dst, src)
                else:
                    nc.scalar.copy(dst, src)
                ev_flip[0] += 1

            # ---- on-chip constants (no DMA) ----
            warm = pc.tile([128, 128], BF16, tag="warm")
            ones16b = pc.tile([128, 16], BF16, tag="ones16b")
            ones128b = pc.tile([128, 1], BF16, tag="ones128b")
            ones1xb = pc.tile([1, 128], BF16, tag="ones1xb")
            epst = pc.tile([1, 1], F32, tag="eps")
            nc.gpsimd.memset(warm[:], 0.0)
            nc.gpsimd.memset(ones16b[:], 1.0)
            nc.gpsimd.memset(ones128b[:], 1.0)
            nc.gpsimd.memset(ones1xb[:], 1.0)
            nc.gpsimd.memset(epst[:], EPS)

            # Pin the activation table to natural_log_exp_and_others up
            # front: it covers every ACT func this kernel uses (exp, ln,
            # square, copy), so the act-table pass inserts no further
            # (1.3us!) table loads mid-chain.
            from concourse.hw_specs import get_activation_tables
            tabs = list(get_activation_tables(nc.m.arch).keys())
            nle_id = tabs.index("natural_log_exp_and_others")
            nc.scalar.add_instruction(
                mybir.InstLoadActFuncSet(
                    name=nc.get_next_instruction_name(),
                    ins=[],
                    outs=[],
                    act_func_set_id=nle_id,
                )
            )

            # PE warm-up: back-to-back dummy transposes keep the tensor
            # engine continuously busy through its p-state ramp while the
            # first x/W chunks stream in.
            wpsum = ptr.tile([128, S], BF16, tag="tr", name="wpsum")
            for _ in range(NWARM):
                nc.tensor.transpose(wpsum[:, 0:128], warm[:], warm[:])

            # ---- DMA'd constants ----
            identb = pc.tile([128, 128], BF16, tag="identb")
            gmatb = pc.tile([128, 32], BF16, tag="gmatb")
            masks = pc.tile([128, 64], F32, tag="masks")
            wsb = pc.tile([128, DCN, NC], BF16, tag="w")

            # ---- stage A: u and uT per batch ----
            # b0's x and W stream in 2-dc chunks, interleaved on one queue so
            # the first matmuls can start after ~2 chunks land.
            us = [[None] * SCN for _ in range(BL)]
            uts = [[None] * KCN for _ in range(BL)]
            xts = []
            for b in range(BL):
                xts.append(px_pool.tile([128, DCN, S], BF16, tag="x",
                                        name=f"xt{b}"))
            for h in range(3):
                nc.sync.dma_start(
                    wsb[:, 2 * h:2 * h + 2, :],
                    W[ds(256 * h, 256), :].rearrange("(dc p) n -> p dc n", p=128),
                )
                nc.sync.dma_start(
                    xts[0][:, 2 * h:2 * h + 2, :],
                    XT[0, ds(256 * h, 256), :].rearrange("(dc p) s -> p dc s", p=128),
                )
            nc.sync.dma_start(identb[:], CB[:, 0:128])
            nc.sync.dma_start(gmatb[:], CB[:, 128:160])
            nc.sync.dma_start(masks[:], MK[:, :])
            for b in range(1, BL):
                nc.sync.dma_start(
                    xts[b][:], XT[b, :, :].rearrange("(dc p) s -> p dc s", p=128)
                )

            for b in range(BL):
                # u tiles [128(s), 512(nc)]: accumulate over dc in dc-major
                # order so b0 consumes x/W chunks as they arrive
                for h in range(2):
                    scs = (2 * h, 2 * h + 1)
                    pus = {sc: pmm.tile([128, NC], F32, tag="mm",
                                        name=f"pu{b}_{sc}")
                           for sc in scs}
                    for dc in range(DCN):
                        for sc in scs:
                            nc.tensor.matmul(
                                pus[sc][:],
                                xts[b][:, dc, ts(sc, 128)],
                                wsb[:, dc, :],
                                start=(dc == 0),
                                stop=(dc == DCN - 1),
                            )
                    for sc in scs:
                        u = pu_pool.tile([128, NC], BF16, tag="u")
                        evac(u[:], pus[sc][:])
                        us[b][sc] = u
                # uT tiles [128(nc), 512(s)] via PE transposes (bf16)
                for kc in range(KCN):
                    put = ptr.tile([128, S], BF16, tag="tr")
                    for sc in range(SCN):
                        nc.tensor.transpose(
                            put[:, ts(sc, 128)],
                            us[b][sc][:, ts(kc, 128)],
                            identb[:],
                        )
                    ut = put_pool.tile([128, S], BF16, tag="ut")
                    evac(ut[:], put[:])
                    uts[b][kc] = ut

            # ---- routing ----
            # cwT[b]: [128(s), (sc n)=64] bf16 softmax weights (it>0)
            cwTs = [None] * BL
            for it in range(ROUTINGS):
                last = it == ROUTINGS - 1
                for b in range(BL):
                    # outputsT: OFT[nc, n] per kc chunk -> pot [128, (kc n)]
                    pot = prm.tile([128, 64], F32, tag="r", name=f"pot{it}_{b}")
                    rhs_full = ones16b if it == 0 else cwTs[b]
                    for kc in range(KCN):
                        for sc in range(SCN):
                            rhs = (rhs_full[:, 0:16] if it == 0
                                   else rhs_full[:, ts(sc, 16)])
                            nc.tensor.matmul(
                                pot[:, ts(kc, 16)],
                                us[b][sc][:, ts(kc, 128)],
                                rhs,
                                start=(sc == 0),
                                stop=(sc == SCN - 1),
                            )
                    # block-diag mask -> mraw; squared norms per capsule
                    mraw = prt.tile([128, 64], F32, tag="mraw")
                    nc.vector.tensor_mul(mraw[:], pot[:], masks[:])
                    sq = prt.tile([128, 64], BF16, tag="sq")
                    nc.scalar.activation(sq[:], mraw[:], AF.Square)
                    pnsq = prm.tile([1, 64], F32, tag="r", name="pnsq")
                    nc.tensor.matmul(pnsq[:], ones128b[:], sq[:],
                                     start=True, stop=True)
                    nred = prt.tile([1, 16], F32, tag="nred")
                    nc.vector.tensor_reduce(
                        nred[:],
                        pnsq[:].rearrange("o (k n) -> o n k", k=4),
                        axis=AX.X,
                        op=OP.add,
                    )
                    # 1/sqrt(nsq+eps) = exp(-0.5*ln(nsq+eps)); both funcs live
                    # in the same act table as softmax's exp
                    lnt = prt.tile([1, 16], F32, tag="lnt")
                    nc.scalar.activation(lnt[:], nred[:], AF.Ln, bias=epst[:])
                    invn = prt.tile([1, 16], BF16, tag="invn")
                    nc.scalar.activation(invn[:], lnt[:], AF.Exp, scale=-0.5)
                    pinv = prm.tile([128, 16], F32, tag="r", name="pinv")
                    nc.tensor.matmul(pinv[:], ones1xb[:], invn[:],
                                     start=True, stop=True)
                    # Mblk = mraw * invnorm (broadcast over the 4 kc groups)
                    pv = pinv[:, :]
                    inv_b = bass.AP(
                        tensor=pv.tensor,
                        offset=pv.offset,
                        ap=[pv.ap[0], [0, 4], [1, 16]],
                    )
                    mblk = prt.tile([128, 64], BF16, tag="mblk")
                    nc.vector.tensor_mul(
                        mblk[:].rearrange("p (k n) -> p k n", k=4),
                        mraw[:].rearrange("p (k n) -> p k n", k=4),
                        inv_b,
                    )
                    if not last:
                        # bT[s, n] per sc -> pbt [128, (sc n)]; softmax over n
                        pbt = prm.tile([128, 64], F32, tag="r",
                                       name=f"pbt{it}_{b}")
                        for sc in range(SCN):
                            for kc in range(KCN):
                                nc.tensor.matmul(
                                    pbt[:, ts(sc, 16)],
                                    uts[b][kc][:, ts(sc, 128)],
                                    mblk[:, ts(kc, 16)],
                                    start=(kc == 0),
                                    stop=(kc == KCN - 1),
                                )
                        expb = prt.tile([128, 64], F32, tag="expb")
                        nc.scalar.activation(expb[:], pbt[:], AF.Exp)
                        zsum = prt.tile([128, 4], F32, tag="zsum")
                        nc.vector.tensor_reduce(
                            zsum[:],
                            expb[:].rearrange("p (g n) -> p g n", g=4),
                            axis=AX.X,
                            op=OP.add,
                        )
                        zrec = prt.tile([128, 4], F32, tag="zrec")
                        nc.vector.reciprocal(zrec[:], zsum[:])
                        zr_ap = zrec[:, :]
                        zr_b = bass.AP(
                            tensor=zr_ap.tensor,
                            offset=zr_ap.offset,
                            ap=[zr_ap.ap[0], [1, 4], [0, 16]],
                        )
                        cwT = prt.tile([128, 64], BF16, tag="cw")
                        nc.vector.tensor_mul(
                            cwT[:].rearrange("p (g n) -> p g n", g=4),
                            expb[:].rearrange("p (g n) -> p g n", g=4),
                            zr_b,
                        )
                        cwTs[b] = cwT
                    else:
                        # final gather: pf[n, c] = sum_kc mblk_kc^T @ gmat
                        pf = prm.tile([16, 32], F32, tag="r", name="pf")
                        for kc in range(KCN):
                            nc.tensor.matmul(
                                pf[:],
                                mblk[:, ts(kc, 16)],
                                gmatb[:],
                                start=(kc == 0),
                                stop=(kc == KCN - 1),
                            )
                        fsb = prt.tile([16, 32], F32, tag="fin")
                        nc.vector.tensor_copy(fsb[:], pf[:])
                        out_eng = nc.sync if b % 2 == 0 else nc.scalar
                        out_eng.dma_start(OUT[b, :, :], fsb[:])

    nc.compile()
    return nc


def _make_consts():
    import ml_dtypes
    bf = ml_dtypes.bfloat16
    cb = np.zeros((128, 160), dtype=bf)
    cb[:, 0:128] = np.eye(128, dtype=np.float32).astype(bf)
    cb[:, 128:160] = np.tile(np.eye(32, dtype=np.float32), (4, 1)).astype(bf)
    masks = np.zeros((128, 64), dtype=np.float32)
    for k in range(4):
        for g in range(4):
            n = 4 * k + g
            masks[32 * g:32 * (g + 1), 16 * k + n] = 1.0
    return {"cb": cb, "mk": masks}


_NC_CACHE = []


def kernel(x: np.ndarray, W: np.ndarray) -> np.ndarray:
    import ml_dtypes
    bf = ml_dtypes.bfloat16
    assert x.shape == (B, S, D) and W.shape == (1, D, NC)
    if not _NC_CACHE:
        _NC_CACHE.append(_build_module())
    nc = _NC_CACHE[0]
    consts = _make_consts()
    w2 = np.ascontiguousarray(W[0]).astype(bf)
    in_maps = []
    for i in range(NCORES):
        m = dict(consts)
        xs = x[i * BL:(i + 1) * BL]
        m["xt"] = np.ascontiguousarray(xs.transpose(0, 2, 1)).astype(bf)
        m["w"] = w2
        in_maps.append(m)
    res = run_bass_kernel_spmd(nc, in_maps, list(range(NCORES)))
    out = np.concatenate([res.results[i]["out"] for i in range(NCORES)], axis=0)
    return out.astype(np.float32)
